# revision 3
# baseline (speedup 1.0000x reference)
"""Trainium2 Bass kernel for nn_AttentionSelectContext.

Reference computation (per batch row b, m=64 context slots, d=256):
    weak  = head_right - head_left
    hW    = weak @ bilinear_w
    score_s[b,m]  = hW[b,:] . rel_s[b,m,:]           (s in {left,right})
    score_s       = where(mask_s, -inf, score_s)
    att_s         = softmax_m(score_s)
    ctx_s[b,:]    = sum_m att_s[b,m] * tail_s[b,m,:]
    y_s           = relu(ctx_s @ w_tail.T + head_s @ w_head.T)
    out_s         = layernorm(y_s + head_s) * gamma + beta

Sharding: data-parallel over batch (4096 -> 8 cores x 512). Weights replicated.

Per-core mapping (P=128 partition tiles, 4 batch tiles):
  - score:   fused multiply-reduce on VectorE (tensor_tensor_reduce per m)
  - softmax: small-tile DVE/ACT ops, exp with fused accumulated denominator
  - weighted tail sum: TensorE diagonal-matmul trick:
        psum += diag(att[:,m]) @ tail_m   (PSUM accumulates over m)
    with diag built on ScalarE: activation(Copy, scale=att[:,m]) of identity.
  - output linears + layernorm: TensorE matmuls (weights pre-transposed via
    PE transpose), DVE/ACT epilogue.
"""

import numpy as np

import concourse.bacc as bacc
import concourse.bass as bass
import concourse.mybir as mybir
import concourse.tile as tile
from concourse.masks import make_identity
from concourse.bass_utils import run_bass_kernel_spmd

N_CORES = 8
B, M, D = 4096, 64, 256
BL = B // N_CORES          # 512 rows per core
P = 128                    # partition tile
NT = BL // P               # 4 batch tiles per core
MC = 32                    # m-chunk per DMA (4 MB transfers)
NMC = M // MC
F32 = mybir.dt.float32
U8 = mybir.dt.uint8
AL = mybir.AluOpType
AF = mybir.ActivationFunctionType
AX = mybir.AxisListType
LN_EPS = 1e-5
NEG_BIG = -1e30

_CACHE = {}

# All ACT functions used here (Copy, Exp, Ln, Relu) live together in the
# "natural_log_exp_and_others" table set, but the table-load pass resolves
# each activation to the FIRST set containing its function — which scatters
# them over three sets and forces a ~1.3us table reload on every switch.
# Putting the all-in-one set first makes every activation resolve to it.
_orig_get_activation_tables = bacc.get_activation_tables


def _patched_get_activation_tables(arch):
    # The emitted act_func_set_id is positional (index into act_info.json's
    # act_func_sets), so the dict order must stay canonical. Instead, strip
    # the functions we use from every set BEFORE natural_log_exp_and_others
    # so first-match resolution picks that one set for all of them.
    tabs = _orig_get_activation_tables(arch)
    key = "natural_log_exp_and_others"
    if key not in tabs:
        return tabs
    mine = {f for f in tabs[key]
            if any(s in str(f) for s in ("Exp", "Ln", "Copy", "Relu",
                                         "Identity", "Square"))}
    out = {}
    seen = False
    for k, funcs in tabs.items():
        if k == key:
            seen = True
            out[k] = funcs
        elif not seen:
            out[k] = {f for f in funcs if f not in mine}
        else:
            out[k] = funcs
    return out


bacc.get_activation_tables = _patched_get_activation_tables


def _build_nc(bl=BL, loops=1, mode="full"):
    nt = bl // P
    nc = bacc.Bacc("TRN2", target_bir_lowering=False, debug=False)

    d_hl = nc.dram_tensor("head_left", [bl, D], F32, kind="ExternalInput").ap()
    d_hr = nc.dram_tensor("head_right", [bl, D], F32, kind="ExternalInput").ap()
    d_rel = {
        "L": nc.dram_tensor("rel_left", [bl, M, D], F32, kind="ExternalInput").ap(),
        "R": nc.dram_tensor("rel_right", [bl, M, D], F32, kind="ExternalInput").ap(),
    }
    d_tail = {
        "L": nc.dram_tensor("tail_left", [bl, M, D], F32, kind="ExternalInput").ap(),
        "R": nc.dram_tensor("tail_right", [bl, M, D], F32, kind="ExternalInput").ap(),
    }
    d_mask = {
        "L": nc.dram_tensor("mask_left", [bl, M], U8, kind="ExternalInput").ap(),
        "R": nc.dram_tensor("mask_right", [bl, M], U8, kind="ExternalInput").ap(),
    }
    d_bw = nc.dram_tensor("bilinear_w", [D, D], F32, kind="ExternalInput").ap()
    d_wt = nc.dram_tensor("w_tail", [D, D], F32, kind="ExternalInput").ap()
    d_wh = nc.dram_tensor("w_head", [D, D], F32, kind="ExternalInput").ap()
    d_g = nc.dram_tensor("ln_gamma", [1, D], F32, kind="ExternalInput").ap()
    d_b = nc.dram_tensor("ln_beta", [1, D], F32, kind="ExternalInput").ap()
    d_out = {
        "L": nc.dram_tensor("out_left", [bl, D], F32, kind="ExternalOutput").ap(),
        "R": nc.dram_tensor("out_right", [bl, D], F32, kind="ExternalOutput").ap(),
    }

    with tile.TileContext(nc) as tc:
        with (
            tc.tile_pool(name="consts", bufs=1) as consts,
            tc.tile_pool(name="relp", bufs=2) as relp,
            tc.tile_pool(name="tailp", bufs=2) as tailp,
            tc.tile_pool(name="work", bufs=2) as work,
            tc.tile_pool(name="diagp", bufs=4) as diagp,
            tc.tile_pool(name="psmm", bufs=2, space="PSUM") as psmm,
            tc.tile_pool(name="pstr", bufs=2, space="PSUM") as pstr,
        ):
            # ---------------- constants ----------------
            ident = consts.tile([P, P], F32, tag="ident")
            make_identity(nc, ident)

            # bilinear_w chunks: rhs[k, d'] with k on partitions (natural layout)
            bw = []
            for c in range(2):
                t = consts.tile([P, D], F32, tag=f"bw{c}")
                nc.sync.dma_start(out=t, in_=d_bw[c * P:(c + 1) * P, :])
                bw.append(t)

            # w_tail^T / w_head^T chunks via PE transpose.
            # wT_c[k_p, j] = W[j, c*128 + k_p]
            def transposed_weight(d_w, label):
                chunks = [
                    consts.tile([P, D], F32, tag=f"{label}T{c}",
                                name=f"{label}T{c}")
                    for c in range(2)
                ]
                for r in range(2):
                    wrow = work.tile([P, D], F32, tag="wrow")
                    nc.sync.dma_start(out=wrow, in_=d_w[r * P:(r + 1) * P, :])
                    for c in range(2):
                        pst = pstr.tile([P, P], F32, tag="tp")
                        nc.tensor.transpose(pst, wrow[:, c * P:(c + 1) * P], ident)
                        nc.scalar.copy(chunks[c][:, r * P:(r + 1) * P], pst)
                return chunks

            wtT = transposed_weight(d_wt, "wt")
            whT = transposed_weight(d_wh, "wh")

            # gamma/beta broadcast across partitions via ones-matmul
            gb = consts.tile([1, 2 * D], F32, tag="gb")
            nc.sync.dma_start(out=gb[:, 0:D], in_=d_g)
            nc.sync.dma_start(out=gb[:, D:2 * D], in_=d_b)
            ones1 = consts.tile([1, P], F32, tag="ones1")
            nc.gpsimd.memset(ones1, 1.0)
            ps_gb = psmm.tile([P, 2 * D], F32, tag="psgb", bufs=1)
            nc.tensor.matmul(ps_gb, lhsT=ones1, rhs=gb, start=True, stop=True)
            gamma_t = consts.tile([P, D], F32, tag="gamma")
            beta_t = consts.tile([P, D], F32, tag="beta")
            nc.scalar.copy(gamma_t, ps_gb[:, 0:D])
            nc.scalar.copy(beta_t, ps_gb[:, D:2 * D])

            # per-tile heads and negative masks, preloaded
            heads = {}
            negmask = {}
            for t in range(nt):
                rows = slice(t * P, (t + 1) * P)
                for key, src in (("L", d_hl), ("R", d_hr)):
                    h = consts.tile([P, D], F32, tag=f"h{key}{t}")
                    nc.sync.dma_start(out=h, in_=src[rows, :])
                    heads[key, t] = h
                for key in ("L", "R"):
                    mk = work.tile([P, M], U8, tag="mk")
                    nc.sync.dma_start(out=mk, in_=d_mask[key][rows, :])
                    nm = consts.tile([P, M], F32, tag=f"nm{key}{t}")
                    nc.vector.tensor_scalar_mul(nm, mk, NEG_BIG)
                    negmask[key, t] = nm

            dummy = consts.tile([P, 1], F32, tag="dummy")
            eps_t = consts.tile([P, 1], F32, tag="eps")
            nc.gpsimd.memset(eps_t, LN_EPS)
            zero_t = consts.tile([P, 1], F32, tag="zero")
            nc.gpsimd.memset(zero_t, 0.0)

            # ---------------- main loop ----------------
            import contextlib
            loop_ctx = (
                tc.For_i(0, loops, 1) if loops > 1 else contextlib.nullcontext()
            )
            with loop_ctx:
                _main_body(nc, tc, nt, heads, negmask, work, relp, tailp, diagp,
                           psmm, pstr, consts, ident, bw, wtT, whT, gamma_t,
                           beta_t, dummy, eps_t, zero_t, d_rel, d_tail, d_out,
                           mode)

    nc.compile()
    return nc


def _main_body(nc, tc, nt, heads, negmask, work, relp, tailp, diagp, psmm,
               pstr, consts, ident, bw, wtT, whT, gamma_t, beta_t, dummy,
               eps_t, zero_t, d_rel, d_tail, d_out, mode="full"):
    do_compute = mode != "dma"
    do_dma = mode != "compute"

    def transpose_pd(src, tag):
        dst = work.tile([P, D], F32, tag=tag, name=tag)
        for c in range(2):
            pst = pstr.tile([P, P], F32, tag="tp", name="pst")
            nc.tensor.transpose(pst, src[:, c * P:(c + 1) * P], ident)
            nc.scalar.copy(dst[:, c * P:(c + 1) * P], pst)
        return dst

    def output_stage(job):
        # deferred by one side: runs after the NEXT side's score has been
        # issued, so DVE never stalls waiting on PE/ACT mid-pipeline
        t, key, ps_out, hT = job
        rows = slice(t * P, (t + 1) * P)
        acc = work.tile([P, D], F32, tag="acc", name="acc")
        nc.scalar.copy(acc, ps_out)

        # y = relu(acc @ w_tail.T + head @ w_head.T)
        accT = transpose_pd(acc, "accT")
        ps_y = psmm.tile([P, D], F32, tag="psy", name="ps_y", bufs=1)
        nc.tensor.matmul(ps_y, lhsT=accT[:, 0:P], rhs=wtT[0],
                         start=True, stop=False)
        nc.tensor.matmul(ps_y, lhsT=accT[:, P:2 * P], rhs=wtT[1],
                         start=False, stop=False)
        nc.tensor.matmul(ps_y, lhsT=hT[:, 0:P], rhs=whT[0],
                         start=False, stop=False)
        nc.tensor.matmul(ps_y, lhsT=hT[:, P:2 * P], rhs=whT[1],
                         start=False, stop=True)
        y = work.tile([P, D], F32, tag="y", name="y")
        nc.scalar.activation(y, ps_y, AF.Relu)

        # layernorm(y + head) * gamma + beta
        z = work.tile([P, D], F32, tag="z", name="z")
        nc.vector.tensor_add(z, y, heads[key, t])
        zsum = work.tile([P, 1], F32, tag="zsum", name="zsum")
        nc.vector.reduce_sum(zsum, z, axis=AX.X)
        negmu = work.tile([P, 1], F32, tag="negmu", name="negmu")
        nc.vector.tensor_scalar_mul(negmu, zsum, -1.0 / D)
        zc = work.tile([P, D], F32, tag="zc", name="zc")
        nc.vector.tensor_scalar_add(zc, z, negmu)
        var = work.tile([P, 1], F32, tag="var", name="var")
        nc.vector.scalar_tensor_tensor(
            dummy.broadcast_to((P, D)), in0=zc, scalar=1.0, in1=zc,
            op0=AL.mult, op1=AL.mult, accum_out=var)
        # rstd = (var/D + eps)^(-1/2) via exp(-0.5*ln(.)) — Ln and Exp share
        # one ACT table set; Sqrt would force a ~1.3us table reload per use
        lnv = work.tile([P, 1], F32, tag="lnv", name="lnv")
        nc.scalar.activation(lnv, var, AF.Ln, bias=eps_t, scale=1.0 / D)
        rstd = work.tile([P, 1], F32, tag="rstd", name="rstd")
        nc.scalar.activation(rstd, lnv, AF.Exp, bias=zero_t, scale=-0.5)
        zg = work.tile([P, D], F32, tag="zg", name="zg")
        nc.vector.scalar_tensor_tensor(
            zg, in0=zc, scalar=rstd, in1=gamma_t, op0=AL.mult, op1=AL.mult)
        zo = work.tile([P, D], F32, tag="zo", name="zo")
        nc.vector.tensor_add(zo, zg, beta_t)
        nc.gpsimd.dma_start(out=d_out[key][rows, :], in_=zo)

    pending = None
    for t in range(nt):
        rows = slice(t * P, (t + 1) * P)
        hl, hr = heads["L", t], heads["R", t]

        if do_compute:
            weak = work.tile([P, D], F32, tag="weak", name="weak")
            nc.vector.tensor_sub(weak, hr, hl)
            weakT = transpose_pd(weak, "weakT")
            headT = {"L": transpose_pd(hl, "hlT"),
                     "R": transpose_pd(hr, "hrT")}

            # hW = weak @ bilinear_w -> [batch, d'], batch on partitions
            ps_hw = psmm.tile([P, D], F32, tag="pshw", name="ps_hw", bufs=1)
            for c in range(2):
                nc.tensor.matmul(ps_hw, lhsT=weakT[:, c * P:(c + 1) * P],
                                 rhs=bw[c], start=(c == 0), stop=(c == 1))
            hW = work.tile([P, D], F32, tag="hW", name="hW")
            nc.scalar.copy(hW, ps_hw)

        for key in ("L", "R"):
            # ---- scores: fused multiply-reduce per m (DVE) ----
            score = work.tile([P, M], F32, tag="score", name="score")
            for mc in range(NMC):
                relc = relp.tile([P, MC, D], F32, tag="relc", name="relc")
                if do_dma:
                    nc.sync.dma_start(
                        out=relc,
                        in_=d_rel[key][rows, mc * MC:(mc + 1) * MC, :])
                else:
                    nc.gpsimd.memset(relc[0:1, 0, 0:2], 0.0)
                if not do_compute:
                    continue
                for ml in range(MC):
                    m = mc * MC + ml
                    nc.vector.scalar_tensor_tensor(
                        dummy.broadcast_to((P, D)), in0=relc[:, ml, :],
                        scalar=1.0, in1=hW, op0=AL.mult, op1=AL.mult,
                        accum_out=score[:, m:m + 1])

            if do_compute:
                # ---- masked softmax over m ----
                scm = work.tile([P, M], F32, tag="scm", name="scm")
                nc.vector.tensor_add(scm, score, negmask[key, t])
                mx = work.tile([P, 1], F32, tag="mx", name="mx")
                nc.vector.reduce_max(mx, scm, axis=AX.X)
                negmx = work.tile([P, 1], F32, tag="negmx", name="negmx")
                nc.vector.tensor_scalar_mul(negmx, mx, -1.0)
                e = work.tile([P, M], F32, tag="e", name="e")
                den = work.tile([P, 1], F32, tag="den", name="den")
                nc.scalar.activation(e, scm, AF.Exp, bias=negmx, scale=1.0,
                                     accum_out=den)
                rs = work.tile([P, 1], F32, tag="rs", name="rs")
                nc.vector.reciprocal(rs, den)
                att = work.tile([P, M], F32, tag="att", name="att")
                nc.vector.tensor_scalar_mul(att, e, rs)
                ps_out = psmm.tile([P, D], F32, tag="psout", name="ps_out")

            # ---- weighted tail sum: psum += diag(att_m) @ tail_m ----
            for mc in range(NMC):
                tailc = tailp.tile([P, MC, D], F32, tag="tailc", name="tailc")
                if do_dma:
                    nc.sync.dma_start(
                        out=tailc,
                        in_=d_tail[key][rows, mc * MC:(mc + 1) * MC, :])
                else:
                    nc.gpsimd.memset(tailc[0:1, 0, 0:2], 0.0)
                if not do_compute:
                    continue
                for ml in range(MC):
                    m = mc * MC + ml
                    dg = diagp.tile([P, P], F32, tag="dg", name="dg")
                    nc.scalar.mul(dg, ident, att[:, m:m + 1])
                    nc.tensor.matmul(ps_out, lhsT=dg, rhs=tailc[:, ml, :],
                                     start=(m == 0), stop=(m == M - 1))

            if not do_compute:
                nc.gpsimd.dma_start(out=d_out[key][rows, :], in_=hl)
                continue

            if pending is not None:
                output_stage(pending)
            pending = (t, key, ps_out, headT[key])

    if pending is not None:
        output_stage(pending)


def _get_nc():
    if "nc" not in _CACHE:
        _CACHE["nc"] = _build_nc()
    return _CACHE["nc"]


def make_in_maps(inputs):
    in_maps = []
    for c in range(N_CORES):
        sl = slice(c * BL, (c + 1) * BL)
        in_maps.append({
            "head_left": np.ascontiguousarray(inputs["head_left"][sl], np.float32),
            "head_right": np.ascontiguousarray(inputs["head_right"][sl], np.float32),
            "rel_left": np.ascontiguousarray(inputs["rel_left"][sl], np.float32),
            "rel_right": np.ascontiguousarray(inputs["rel_right"][sl], np.float32),
            "tail_left": np.ascontiguousarray(inputs["tail_left"][sl], np.float32),
            "tail_right": np.ascontiguousarray(inputs["tail_right"][sl], np.float32),
            "mask_left": np.ascontiguousarray(
                inputs["mask_left"][sl].astype(np.uint8)),
            "mask_right": np.ascontiguousarray(
                inputs["mask_right"][sl].astype(np.uint8)),
            "bilinear_w": np.ascontiguousarray(inputs["bilinear_w"], np.float32),
            "w_tail": np.ascontiguousarray(inputs["w_tail"], np.float32),
            "w_head": np.ascontiguousarray(inputs["w_head"], np.float32),
            "ln_gamma": np.ascontiguousarray(
                inputs["ln_gamma"], np.float32).reshape(1, D),
            "ln_beta": np.ascontiguousarray(
                inputs["ln_beta"], np.float32).reshape(1, D),
        })
    return in_maps


def kernel(**inputs):
    nc = _get_nc()
    in_maps = make_in_maps(inputs)
    res = run_bass_kernel_spmd(nc, in_maps, list(range(N_CORES))).results
    left = np.concatenate([res[c]["out_left"] for c in range(N_CORES)], axis=0)
    right = np.concatenate([res[c]["out_right"] for c in range(N_CORES)], axis=0)
    return (left, right)

